# revision 12
# baseline (speedup 1.0000x reference)
"""DeepSpeedSelfAttention (LN + QKV + softmax-attention + out-proj) on 8 trn2 cores.

Sharding: core c -> (batch b = c//2, head-group g = c%2 of 8 heads).
Each core computes, for its batch and its 8 heads:
  - LayerNorm of the full sequence (norm_w/norm_b folded into weights on host)
  - Q^T,K^T (feature-major) and V (token-major) projections in bf16
  - scores^T = K^T.T @ Q^T per head (transposed so softmax-k lands on partitions)
  - exp on ACT; denominator via an augmented 65th V column (= exp(mask) per key)
  - ctx^T accumulated over key tiles; normalized by the broadcast reciprocal
  - partial output projection (host sums the two per-batch partials)
Outputs per core: key/value/context slices (token-major f32) + out partial.
"""

import sys

for _p in ("/opt/trn_rl_repo", "/opt/trn_rl_repo/concourse"):
    if _p not in sys.path:
        sys.path.insert(0, _p)

import numpy as np
import ml_dtypes

import concourse.bass as bass
import concourse.tile as tile
from concourse import mybir, bacc
from concourse import bass_utils
from concourse.bass import ts

F32 = mybir.dt.float32
BF16 = mybir.dt.bfloat16
AF = mybir.ActivationFunctionType
ALU = mybir.AluOpType

B, S, H = 4, 2048, 1024
HEADS = 16
DH = H // HEADS          # 64
N_CORES = 8
HPC = HEADS // 2         # 8 heads per core
RW = HPC * DH            # 512 features per core
EPS = 1e-12
TT_N = S // 128          # 16 token tiles
DAUG = DH + 1            # 65: v features + denominator column


def build_program():
    nc = bacc.Bacc(trn_type="TRN2")

    x_d = nc.dram_tensor("x", [S, H], F32, kind="ExternalInput")
    wqT_d = nc.dram_tensor("wqT", [128, 8, RW], BF16, kind="ExternalInput")
    wkT_d = nc.dram_tensor("wkT", [128, 8, RW], BF16, kind="ExternalInput")
    wvT_d = nc.dram_tensor("wvT", [128, 8, RW], BF16, kind="ExternalInput")
    owT_d = nc.dram_tensor("owT", [128, 4, H], BF16, kind="ExternalInput")
    bq_d = nc.dram_tensor("bq", [128, 4], F32, kind="ExternalInput")
    bk_d = nc.dram_tensor("bk", [128, 4], F32, kind="ExternalInput")
    bvb_d = nc.dram_tensor("bvb", [128, RW], F32, kind="ExternalInput")
    expm_d = nc.dram_tensor("expm", [128, TT_N], F32, kind="ExternalInput")

    key_d = nc.dram_tensor("key_out", [S, RW], F32, kind="ExternalOutput")
    val_d = nc.dram_tensor("value_out", [S, RW], F32, kind="ExternalOutput")
    ctx_d = nc.dram_tensor("ctx_out", [S, RW], F32, kind="ExternalOutput")
    out_d = nc.dram_tensor("out_partial", [S, H], F32, kind="ExternalOutput")

    x_ap, key_ap, val_ap, ctx_ap, out_ap = (
        x_d.ap(), key_d.ap(), val_d.ap(), ctx_d.ap(), out_d.ap())

    with tile.TileContext(nc) as tc:
        with (
            tc.tile_pool(name="const", bufs=1) as cp,
            tc.tile_pool(name="persist", bufs=1) as pp,
        ):
            wq_s = cp.tile([128, 8, RW], BF16)
            nc.sync.dma_start(out=wq_s, in_=wqT_d.ap())
            wk_s = cp.tile([128, 8, RW], BF16)
            nc.sync.dma_start(out=wk_s, in_=wkT_d.ap())
            wv_s = cp.tile([128, 8, RW], BF16)
            nc.sync.dma_start(out=wv_s, in_=wvT_d.ap())
            ow_s = cp.tile([128, 4, H], BF16)
            nc.sync.dma_start(out=ow_s, in_=owT_d.ap())
            bq_s = cp.tile([128, 4], F32)
            nc.sync.dma_start(out=bq_s, in_=bq_d.ap())
            bk_s = cp.tile([128, 4], F32)
            nc.sync.dma_start(out=bk_s, in_=bk_d.ap())
            bvb_s = cp.tile([128, RW], F32)
            nc.sync.dma_start(out=bvb_s, in_=bvb_d.ap())
            expm_s = cp.tile([128, TT_N], F32)
            nc.sync.dma_start(out=expm_s, in_=expm_d.ap())
            eps_s = cp.tile([128, 1], F32)
            nc.vector.memset(eps_s, EPS)

            QT = pp.tile([128, 4, S], BF16)   # [r%128, rc, t] q features
            KT = pp.tile([128, 4, S], BF16)
            Vg = pp.tile([128, TT_N, HPC, DAUG], BF16)  # [t%128, tt, head, 64v+denomcol]
            ctxT = pp.tile([128, 4, S], BF16)  # [j%128, jc, t]
            key_tm = pp.tile([128, TT_N, RW], BF16)  # [t%128, tt, r] token-major
            ctx_tm = pp.tile([128, TT_N, RW], BF16)
            nc.vector.memset(Vg, 0.0)

            # ---------------- Phase A+B: LN, lnT, QKV ----------------
            with (
                tc.tile_pool(name="lnTp", bufs=1) as lnTp,
                tc.tile_pool(name="lnw", bufs=3) as lw,
                tc.tile_pool(name="psq", bufs=2, space="PSUM") as psq,
            ):
                lnT4 = [lnTp.tile([128, 8, S // 4], BF16, tag=f"lnT{q}", name=f"lnT{q}")
                        for q in range(4)]  # [h%128, hc, t-quarter]
                for tt in range(TT_N):
                    xt = lw.tile([128, H], F32, tag="xt")
                    nc.sync.dma_start(out=xt, in_=x_ap[ts(tt, 128), :])
                    stats = lw.tile([128, 2, 6], F32, tag="st")
                    nc.vector.bn_stats(out=stats[:, 0, :], in_=xt[:, 0:512])
                    nc.vector.bn_stats(out=stats[:, 1, :], in_=xt[:, 512:1024])
                    mv = lw.tile([128, 2], F32, tag="mv")
                    nc.vector.bn_aggr(out=mv, in_=stats)
                    sd = lw.tile([128, 1], F32, tag="sd")
                    nc.scalar.activation(out=sd, in_=mv[:, 1:2], func=AF.Sqrt,
                                         bias=eps_s[:, 0:1], scale=1.0)
                    rstd = lw.tile([128, 1], F32, tag="rstd")
                    nc.vector.reciprocal(out=rstd, in_=sd)
                    lnt = lw.tile([128, H], BF16, tag="lnt")
                    nc.vector.tensor_scalar(out=lnt, in0=xt, scalar1=mv[:, 0:1],
                                            scalar2=rstd, op0=ALU.subtract,
                                            op1=ALU.mult)
                    nc.sync.dma_start(
                        out=lnT4[tt // 4][:, :, ts(tt % 4, 128)], in_=lnt,
                        transpose=True)

                # V token-major: psum[t 128, rv 512] = lnT[h, t].T @ WvT[h, rv]
                for tt in range(TT_N):
                    ps = psq.tile([128, RW], F32, tag="qkv")
                    for hc in range(8):
                        nc.tensor.matmul(
                            ps, lnT4[tt // 4][:, hc, ts(tt % 4, 128)],
                            wv_s[:, hc, :],
                            start=(hc == 0), stop=(hc == 7))
                    v1 = lw.tile([128, RW], F32, tag="v1")
                    nc.vector.tensor_add(out=v1, in0=ps, in1=bvb_s)
                    nc.sync.dma_start(out=val_ap[ts(tt, 128), :], in_=v1)
                    v1h = v1[:].rearrange("p (h d) -> p h d", d=DH)
                    nc.vector.tensor_scalar_mul(
                        out=Vg[:, tt, :, 0:DH], in0=v1h,
                        scalar1=expm_s[:, tt:tt + 1])
                    # denominator column = exp(mask_k): 0 + expm
                    nc.vector.tensor_scalar_add(
                        out=Vg[:, tt, :, DH:DAUG], in0=Vg[:, tt, :, DH:DAUG],
                        scalar1=expm_s[:, tt:tt + 1])

                # Q^T / K^T: psum[r 128, t 512] groups; 2-bank psq pool so
                # attention's 6 banks can coexist (QKV overlaps attention)
                for rc in range(4):
                    for (w_s, b_s, dst) in ((wq_s, bq_s, QT), (wk_s, bk_s, KT)):
                        for tb in range(4):
                            ps = psq.tile([128, 512], F32, tag="qkv")
                            for hc in range(8):
                                nc.tensor.matmul(
                                    ps, w_s[:, hc, ts(rc, 128)],
                                    lnT4[tb][:, hc, :],
                                    start=(hc == 0), stop=(hc == 7))
                            nc.vector.tensor_scalar_add(
                                out=dst[:, rc, ts(tb, 512)], in0=ps,
                                scalar1=b_s[:, rc:rc + 1])

            # key_out: 4 wide DMA transposes KT -> token-major, then cast+store
            with tc.tile_pool(name="kw", bufs=3) as kw:
                for rc in range(4):
                    nc.sync.dma_start(out=key_tm[:, :, ts(rc, 128)],
                                      in_=KT[:, rc, :], transpose=True)
                for tt in range(TT_N):
                    kst = kw.tile([128, RW], F32, tag="kst")
                    nc.vector.tensor_copy(out=kst, in_=key_tm[:, tt, :])
                    nc.sync.dma_start(out=key_ap[ts(tt, 128), :], in_=kst)

            # ---------------- Phase C: attention ----------------
            with (
                tc.tile_pool(name="pss", bufs=2, space="PSUM") as pss,
                tc.tile_pool(name="psc", bufs=1, space="PSUM") as psc,
                tc.tile_pool(name="aw", bufs=3) as aw,
            ):
                for hp in range(4):
                    h0, h1 = 2 * hp, 2 * hp + 1
                    for qq in range(4):
                        pctx0 = psc.tile([DAUG, 512], F32, tag="c0")
                        pctx1 = psc.tile([DAUG, 512], F32, tag="c1")
                        for kt in range(TT_N):
                            ps_s = pss.tile([128, 2, 512], F32, tag="s")
                            nc.tensor.matmul(
                                ps_s[:, 0, :], KT[0:64, hp, ts(kt, 128)],
                                QT[0:64, hp, ts(qq, 512)],
                                start=True, stop=True)
                            nc.tensor.matmul(
                                ps_s[:, 1, :], KT[64:128, hp, ts(kt, 128)],
                                QT[64:128, hp, ts(qq, 512)],
                                start=True, stop=True, tile_position=(64, 0))
                            pt = aw.tile([128, 2, 512], BF16, tag="pt")
                            nc.scalar.activation(out=pt, in_=ps_s, func=AF.Exp,
                                                 scale=0.125)
                            nc.tensor.matmul(pctx0, Vg[:, kt, h0, :],
                                             pt[:, 0, :], start=(kt == 0),
                                             stop=(kt == TT_N - 1))
                            nc.tensor.matmul(pctx1, Vg[:, kt, h1, :],
                                             pt[:, 1, :], start=(kt == 0),
                                             stop=(kt == TT_N - 1))
                        for j, pctx in ((0, pctx0), (1, pctx1)):
                            den = aw.tile([1, 512], F32, tag="den")
                            nc.vector.tensor_copy(out=den, in_=pctx[DH:DAUG, :])
                            rec = aw.tile([1, 512], F32, tag="rec")
                            nc.vector.reciprocal_approx_fast(out=rec, in_=den)
                            rbc = aw.tile([64, 512], F32, tag="rbc")
                            nc.gpsimd.partition_broadcast(rbc, rec)
                            nc.vector.tensor_mul(
                                out=ctxT[64 * j:64 * (j + 1), hp, ts(qq, 512)],
                                in0=pctx[0:DH, :], in1=rbc)
                    if qq == 3:
                        nc.sync.dma_start(out=ctx_tm[:, :, ts(hp, 128)],
                                          in_=ctxT[:, hp, :], transpose=True)
                        cstw = aw.tile([128, TT_N, 128], F32, tag="cstw")
                        nc.vector.tensor_copy(out=cstw,
                                              in_=ctx_tm[:, :, ts(hp, 128)])
                        ctx_col = ctx_ap[:, ts(hp, 128)].rearrange(
                            "(tt p) r -> p tt r", p=128)
                        nc.sync.dma_start(out=ctx_col, in_=cstw)

            # ---------------- Phase D: out projection ----------------
            with (
                tc.tile_pool(name="pso", bufs=2, space="PSUM") as pso,
                tc.tile_pool(name="ow", bufs=3) as owp,
            ):
                for tt in range(TT_N):
                    ost = owp.tile([128, H], F32, tag="ost")
                    ps = pso.tile([128, 2, 512], F32, tag="o")
                    for jc in range(4):
                        for ib in range(2):
                            nc.tensor.matmul(ps[:, ib, :],
                                             ctxT[:, jc, ts(tt, 128)],
                                             ow_s[:, jc, ts(ib, 512)],
                                             start=(jc == 0), stop=(jc == 3))
                    for ib in range(2):
                        nc.vector.tensor_copy(out=ost[:, ts(ib, 512)],
                                              in_=ps[:, ib, :])
                    nc.sync.dma_start(out=out_ap[ts(tt, 128), :], in_=ost)



    nc.compile()
    return nc


def host_prep(inputs):
    """Build the 8 per-core input maps from full inputs."""
    x = np.asarray(inputs["input"], np.float32)
    mask = np.asarray(inputs["input_mask"], np.float32)
    norm_w = np.asarray(inputs["norm_w"], np.float32)
    norm_b = np.asarray(inputs["norm_b"], np.float32)
    qkvw = np.asarray(inputs["attn_qkvw"], np.float32)
    qkvb = np.asarray(inputs["attn_qkvb"], np.float32)
    ow = np.asarray(inputs["attn_ow"], np.float32)

    wfold = qkvw * norm_w[None, :]          # [3H, H]
    bfold = qkvb + qkvw @ norm_b            # [3H]

    in_maps = []
    for c in range(N_CORES):
        b, g = divmod(c, 2)
        rq = slice(g * RW, g * RW + RW)
        rk = slice(H + g * RW, H + g * RW + RW)
        rv = slice(2 * H + g * RW, 2 * H + g * RW + RW)

        def wT(rows):
            # [RW, H] -> W^T [H, RW] -> [128, 8, RW] with h = hc*128 + p
            w = wfold[rows].T.astype(ml_dtypes.bfloat16)
            return np.ascontiguousarray(
                w.reshape(8, 128, RW).transpose(1, 0, 2))

        def bcol(rows):
            # [RW] -> [128, 4] with r = rc*128 + p
            return np.ascontiguousarray(
                bfold[rows].reshape(4, 128).T.astype(np.float32))

        owT = ow[:, g * RW:g * RW + RW].T.astype(ml_dtypes.bfloat16)  # [RW, H]
        owT = np.ascontiguousarray(owT.reshape(4, 128, H).transpose(1, 0, 2))
        expm = np.exp(mask[b, 0, 0, :]).astype(np.float32)
        in_maps.append({
            "x": np.ascontiguousarray(x[b]),
            "wqT": wT(rq), "wkT": wT(rk), "wvT": wT(rv),
            "owT": owT,
            "bq": bcol(rq), "bk": bcol(rk),
            "bvb": np.ascontiguousarray(
                np.broadcast_to(bfold[rv][None, :], (128, RW)).astype(np.float32)),
            "expm": np.ascontiguousarray(expm.reshape(TT_N, 128).T),
        })
    return in_maps


_NC_CACHE = None


def run_cores(in_maps, **kwargs):
    global _NC_CACHE
    if _NC_CACHE is None:
        _NC_CACHE = build_program()
    return bass_utils.run_bass_kernel_spmd(
        _NC_CACHE, in_maps, core_ids=list(range(N_CORES)), **kwargs)


def assemble(results):
    out = np.zeros((B, S, H), np.float32)
    key = np.zeros((B, S, H), np.float32)
    val = np.zeros((B, S, H), np.float32)
    ctx = np.zeros((B, S, H), np.float32)
    for c in range(N_CORES):
        b, g = divmod(c, 2)
        cols = slice(g * RW, g * RW + RW)
        r = results[c]
        key[b, :, cols] = r["key_out"]
        val[b, :, cols] = r["value_out"]
        ctx[b, :, cols] = r["ctx_out"]
        out[b] += r["out_partial"]
    return out, key, val, ctx


def kernel(**inputs):
    in_maps = host_prep(inputs)
    res = run_cores(in_maps)
    return assemble(res.results)
